# revision 11
# baseline (speedup 1.0000x reference)
"""GCN (2-layer GCNConv + global mean pool) on 8 Trainium2 NeuronCores.

Strategy:
  out = pool( relu(A' relu(A' X W1 + b1) W2 + b2) ), A' = D^-1/2 (A+I) D^-1/2.
  Normalization factors out: A' H = dinv * ((A+I) (dinv * H)).
  - Host: build per-core edge lists (sharded by dst-node range), sorted by dst,
    packed into windows of <=128 destination nodes x (2 src-halves x TPH tiles
    of 128 gather slots).  Self-loops are plain edges.  Gather indices int16
    (tables split into two 25000-row halves).
  - Device per core: dma_gather rows of the dinv-scaled node table, segment-sum
    via one-hot is_equal + PE matmul into PSUM, epilogue = dinv-scale, PE
    transpose, matmul by W (bias via an all-ones feature slot), relu,
    indirect-scatter into this core's slice of the next layer's table,
    AllGather slices between layers, one-hot matmul graph pooling at the end.
  - Host: sum per-core pooled partials, divide by graph sizes.
"""
import numpy as np

N = 50000
D = 133
DP = 192           # padded feature dim (192 f32 = 768B per gather row)
G = 256            # graphs
NC = 8
NLOC = N // NC     # 6250 nodes per core
HALF = N // 2      # gather-table half size (int16-indexable)
TPH = 8            # gather tiles per half-window (dma_gather limit: 1024 idxs/call)
CAP = TPH * 128    # src slots per half-window
BIG = 1 << 20      # OOB scatter index (dropped by bounds_check)

_prog_cache = {}


def _pack_core(es, ed):
    """Pack one core's dst-sorted edges into windows.

    es: global src ids, ed: local dst ids (0..NLOC), both sorted by ed.
    Returns windows [(n0, n1, sA, dA, sB, dB)]: node range [n0,n1), per-half
    src ids and local dst ids (window edge lists, dst-sorted).
    """
    is_b = es >= HALF
    eA, dA = es[~is_b], ed[~is_b]
    eB, dB = es[is_b], ed[is_b]
    cumA = np.concatenate([[0], np.cumsum(np.bincount(dA, minlength=NLOC))])
    cumB = np.concatenate([[0], np.cumsum(np.bincount(dB, minlength=NLOC))])
    windows = []
    n0 = 0
    while n0 < NLOC:
        n1 = min(n0 + 128, NLOC)
        a_hi = int(np.searchsorted(cumA, cumA[n0] + CAP, side="right")) - 1
        b_hi = int(np.searchsorted(cumB, cumB[n0] + CAP, side="right")) - 1
        n1 = min(n1, a_hi, b_hi)
        if n1 <= n0:
            raise RuntimeError(f"node {n0} degree exceeds window capacity")
        windows.append((n0, n1,
                        eA[cumA[n0]:cumA[n1]], dA[cumA[n0]:cumA[n1]],
                        eB[cumB[n0]:cumB[n1]], dB[cumB[n0]:cumB[n1]]))
        n0 = n1
    return windows


def _wrap16(a):
    """[W, CAP] int16 -> [128, W*CAP/16] per-16 wrap, replicated x8."""
    Wn = a.shape[0]
    w16 = a.reshape(Wn, CAP // 16, 16).transpose(2, 0, 1).reshape(16, -1)
    return np.tile(w16, (8, 1)).copy()


def preprocess(x, edge_index, batch, W1, b1, W2, b2):
    src = np.asarray(edge_index[0], dtype=np.int64)
    dst = np.asarray(edge_index[1], dtype=np.int64)
    deg = np.bincount(dst, minlength=N).astype(np.float64) + 1.0
    dinv = (1.0 / np.sqrt(deg)).astype(np.float32)

    loop = np.arange(N, dtype=np.int64)          # self-loops as plain edges
    srcs = np.concatenate([src, loop])
    dsts = np.concatenate([dst, loop])

    x1p = np.zeros((N, DP), np.float32)
    x1p[:, :D] = dinv[:, None] * np.asarray(x, np.float32)

    batch_np = np.asarray(batch, np.int64)
    per_core_wins = []
    for k in range(NC):
        base = k * NLOC
        m = (dsts >= base) & (dsts < base + NLOC)
        es = srcs[m]
        ed = (dsts[m] - base).astype(np.int64)
        order = np.argsort(ed, kind="stable")
        per_core_wins.append(_pack_core(es[order], ed[order]))

    W = max(len(w) for w in per_core_wins)

    cores = []
    for k in range(NC):
        base = k * NLOC
        wins = per_core_wins[k]
        idxA = np.zeros((W, CAP), np.int16)
        idxB = np.zeros((W, CAP), np.int16)
        dstloc = np.full((W, 2 * CAP), -1.0, np.float32)
        sidx = np.full((W, 128), BIG, np.int32)
        dinvw = np.ones((W, 128), np.float32)
        batchg = np.full((W, 128), -1.0, np.float32)
        for w, (n0, n1, sA, dA, sB, dB) in enumerate(wins):
            nn = n1 - n0
            idxA[w, :len(sA)] = sA.astype(np.int16)
            idxB[w, :len(sB)] = (sB - HALF).astype(np.int16)
            dstloc[w, :len(dA)] = (dA - n0).astype(np.float32)
            dstloc[w, CAP:CAP + len(dB)] = (dB - n0).astype(np.float32)
            sidx[w, :nn] = np.arange(n0, n1, dtype=np.int32)
            dinvw[w, :nn] = dinv[base + np.arange(n0, n1)]
            batchg[w, :nn] = batch_np[base + np.arange(n0, n1)].astype(np.float32)
        # device layouts
        cores.append(dict(
            idxa=_wrap16(idxA),
            idxb=_wrap16(idxB),
            # slot i of window tile t -> [i%128, w*2*TPH + t]
            dstloc=dstloc.reshape(W, 2 * TPH, 128).transpose(2, 0, 1).reshape(128, W * 2 * TPH).copy(),
            sidx=sidx.T.copy(),          # [128, W]
            dinvw=dinvw.T.copy(),        # [128, W]
            batchg=batchg.T.copy(),      # [128, W]
        ))

    def wpad(Wm, bv):
        p = np.zeros((DP, DP), np.float32)
        p[:D, :D] = np.asarray(Wm, np.float32)
        p[DP - 1, :D] = np.asarray(bv, np.float32)   # bias via ones-slot 191
        return p[:128].copy(), p[128:].copy()        # [128,192], [64,192]

    wa1, wb1 = wpad(W1, b1)
    wa2, wb2 = wpad(W2, b2)
    consts = dict(
        iota=np.tile(np.arange(G, dtype=np.float32), (128, 1)),
        ident=np.eye(128, dtype=np.float32),
        wa1=wa1, wb1=wb1, wa2=wa2, wb2=wb2, x1p=x1p,
    )
    counts = np.bincount(batch_np, minlength=G).astype(np.float32)
    return cores, consts, W, counts


def build_program(W, use_collective=True):
    import concourse.bacc as bacc
    import concourse.bass as bass
    import concourse.mybir as mybir
    import concourse.tile as tile

    nc = bacc.Bacc("TRN2", target_bir_lowering=False, debug=False)
    dt = mybir.dt
    f32 = dt.float32

    x1p_d = nc.dram_tensor("x1p", [N, DP], f32, kind="ExternalInput")
    idxa_d = nc.dram_tensor("idxa", [128, W * CAP // 16], dt.int16, kind="ExternalInput")
    idxb_d = nc.dram_tensor("idxb", [128, W * CAP // 16], dt.int16, kind="ExternalInput")
    dstloc_d = nc.dram_tensor("dstloc", [128, W * 2 * TPH], f32, kind="ExternalInput")
    sidx_d = nc.dram_tensor("sidx", [128, W], dt.int32, kind="ExternalInput")
    dinvw_d = nc.dram_tensor("dinvw", [128, W], f32, kind="ExternalInput")
    batchg_d = nc.dram_tensor("batchg", [128, W], f32, kind="ExternalInput")
    iota_d = nc.dram_tensor("iota", [128, G], f32, kind="ExternalInput")
    ident_d = nc.dram_tensor("ident", [128, 128], f32, kind="ExternalInput")
    w_d = {nm: nc.dram_tensor(nm, [128 if nm[1] == "a" else 64, DP], f32,
                              kind="ExternalInput")
           for nm in ("wa1", "wb1", "wa2", "wb2")}
    pool_out = nc.dram_tensor("pool", [G, DP], f32, kind="ExternalOutput")

    t2full = nc.dram_tensor("t2full", [N, DP], f32)
    t2b = nc.dram_tensor("t2b", [NLOC, DP], f32)

    Relu = mybir.ActivationFunctionType.Relu
    Copy = mybir.ActivationFunctionType.Copy

    with tile.TileContext(nc) as tc:
        with (
            tc.tile_pool(name="const", bufs=1) as cpool,
            tc.tile_pool(name="work", bufs=3) as wpool,
            tc.tile_pool(name="oh", bufs=4) as ohpool,
            tc.tile_pool(name="ps_agg", bufs=2, space="PSUM") as ps_agg,
            tc.tile_pool(name="ps_tp", bufs=1, space="PSUM") as ps_tp,
            tc.tile_pool(name="ps_out", bufs=2, space="PSUM") as ps_out,
            tc.tile_pool(name="ps_pool", bufs=1, space="PSUM") as ps_pool,
        ):
            def cload(dram, shape, dtype=f32):
                t = cpool.tile(shape, dtype, name=f"c_{dram.name}",
                               tag=f"c_{dram.name}")
                nc.sync.dma_start(out=t[:], in_=dram[:])
                return t

            idxa = cload(idxa_d, [128, W * CAP // 16], dt.int16)
            idxb = cload(idxb_d, [128, W * CAP // 16], dt.int16)
            dstloc = cload(dstloc_d, [128, W * 2 * TPH])
            sidx = cload(sidx_d, [128, W], dt.int32)
            dinvw = cload(dinvw_d, [128, W])
            batchg = cload(batchg_d, [128, W])
            iota = cload(iota_d, [128, G])
            ident = cload(ident_d, [128, 128])
            wt = {nm: cload(w_d[nm], [128 if nm[1] == "a" else 64, DP])
                  for nm in ("wa1", "wb1", "wa2", "wb2")}

            pool_ps = [ps_pool.tile([128, DP], f32, space="PSUM", tag=f"pp{i}",
                                    name=f"pool_ps{i}")
                       for i in range(2)]

            for lam in (0, 1):
                tab = x1p_d if lam == 0 else t2full
                wa = wt["wa1"] if lam == 0 else wt["wa2"]
                wb = wt["wb1"] if lam == 0 else wt["wb2"]
                for w in range(W):
                    agg = ps_agg.tile([128, DP], f32, space="PSUM", tag="agg")
                    for h in (0, 1):
                        msg = wpool.tile([128, TPH, DP], f32, tag="msg")
                        idx_t = idxa if h == 0 else idxb
                        tab_ap = tab[0:HALF, :] if h == 0 else tab[HALF:N, :]
                        nc.gpsimd.dma_gather(
                            msg[:], tab_ap,
                            idx_t[:, w * (CAP // 16):(w + 1) * (CAP // 16)],
                            CAP, CAP, DP,
                        )
                        for t in range(TPH):
                            oh = ohpool.tile([128, 128], f32, tag="oh")
                            col = w * 2 * TPH + h * TPH + t
                            nc.vector.tensor_tensor(
                                out=oh[:],
                                in0=dstloc[:, col:col + 1].to_broadcast([128, 128]),
                                in1=iota[:, 0:128],
                                op=mybir.AluOpType.is_equal,
                            )
                            nc.tensor.matmul(
                                out=agg[:], lhsT=oh[:], rhs=msg[:, t, :],
                                start=(h == 0 and t == 0),
                                stop=(h == 1 and t == TPH - 1),
                            )
                    sT = wpool.tile([128, DP], f32, tag="sT")
                    nc.scalar.activation(out=sT[:], in_=agg[:], func=Copy,
                                         scale=dinvw[:, w:w + 1])
                    nc.vector.memset(sT[:, DP - 1:DP], 1.0)
                    tp1 = ps_tp.tile([128, 128], f32, space="PSUM", tag="tp1")
                    tp2 = ps_tp.tile([64, 128], f32, space="PSUM", tag="tp2")
                    nc.tensor.transpose(out=tp1[:], in_=sT[:, 0:128],
                                        identity=ident[:])
                    nc.tensor.transpose(out=tp2[:], in_=sT[:, 128:DP],
                                        identity=ident[:])
                    tT1 = wpool.tile([128, 128], f32, tag="tT1")
                    tT2 = wpool.tile([64, 128], f32, tag="tT2")
                    nc.scalar.activation(out=tT1[:], in_=tp1[:], func=Copy)
                    nc.scalar.activation(out=tT2[:], in_=tp2[:], func=Copy)
                    outp = ps_out.tile([128, DP], f32, space="PSUM", tag="outp")
                    nc.tensor.matmul(out=outp[:], lhsT=tT1[:], rhs=wa[:], start=True, stop=False)
                    nc.tensor.matmul(out=outp[:], lhsT=tT2[:], rhs=wb[:], start=False, stop=True)
                    if lam == 0:
                        tabt = wpool.tile([128, DP], f32, tag="tabt")
                        nc.scalar.activation(out=tabt[:], in_=outp[:], func=Relu,
                                             scale=dinvw[:, w:w + 1])
                        nc.gpsimd.indirect_dma_start(
                            out=t2b[:],
                            out_offset=bass.IndirectOffsetOnAxis(
                                ap=sidx[:, w:w + 1], axis=0),
                            in_=tabt[:],
                            in_offset=None,
                            bounds_check=NLOC - 1,
                            oob_is_err=False,
                        )
                    else:
                        h2 = wpool.tile([128, DP], f32, tag="tabt")
                        nc.scalar.activation(out=h2[:], in_=outp[:], func=Relu)
                        og = ohpool.tile([128, G], f32, tag="og")
                        nc.vector.tensor_tensor(
                            out=og[:],
                            in0=batchg[:, w:w + 1].to_broadcast([128, G]),
                            in1=iota[:],
                            op=mybir.AluOpType.is_equal,
                        )
                        for i in range(2):
                            nc.tensor.matmul(
                                out=pool_ps[i][:],
                                lhsT=og[:, 128 * i:128 * (i + 1)], rhs=h2[:],
                                start=(w == 0), stop=(w == W - 1),
                            )
                if lam == 0 and use_collective:
                    nc.gpsimd.collective_compute(
                        "AllGather", mybir.AluOpType.bypass,
                        replica_groups=[list(range(NC))],
                        ins=[t2b[:]], outs=[t2full[:]],
                    )
            for i in range(2):
                po = wpool.tile([128, DP], f32, tag="po")
                nc.scalar.activation(out=po[:], in_=pool_ps[i][:], func=Copy)
                nc.sync.dma_start(out=pool_out[128 * i:128 * (i + 1), :], in_=po[:])

    nc.compile()
    return nc


def kernel(**inputs):
    from concourse.bass_utils import run_bass_kernel_spmd

    cores, consts, W, counts = preprocess(**inputs)
    if W not in _prog_cache:
        _prog_cache[W] = build_program(W)
    nc = _prog_cache[W]

    in_maps = [{**consts, **{k2: v for k2, v in c.items()}} for c in cores]
    res = run_bass_kernel_spmd(nc, in_maps, core_ids=list(range(NC)))
    total = np.zeros((G, DP), np.float32)
    for c in range(NC):
        total += res.results[c]["pool"]
    out = total[:, :D] / np.maximum(counts, 1.0)[:, None]
    return out.astype(np.float32)


# revision 13
# speedup vs baseline: 62.2870x; 62.2870x over previous
"""GCN (2-layer GCNConv + global mean pool) on 8 Trainium2 NeuronCores.

Strategy:
  out = pool( relu(A' relu(A' X W1 + b1) W2 + b2) ), A' = D^-1/2 (A+I) D^-1/2.
  Normalization factors out: A' H = dinv * ((A+I) (dinv * H)).
  - Host: build per-core edge lists (sharded by dst-node range), sorted by dst,
    packed into windows of <=128 destination nodes x (2 src-halves x TPH tiles
    of 128 gather slots).  Self-loops are plain edges.  Gather indices int16
    (tables split into two 25000-row halves).
  - Device per core: dma_gather rows of the dinv-scaled node table, segment-sum
    via one-hot is_equal + PE matmul into PSUM, epilogue = dinv-scale, PE
    transpose, matmul by W (bias via an all-ones feature slot), relu,
    indirect-scatter into this core's slice of the next layer's table,
    AllGather slices between layers, one-hot matmul graph pooling at the end.
  - Host: sum per-core pooled partials, divide by graph sizes.
"""
import numpy as np

N = 50000
D = 133
DP = 192           # padded feature dim (192 f32 = 768B per gather row)
G = 256            # graphs
NC = 8
NLOC = N // NC     # 6250 nodes per core
HALF = N // 2      # gather-table half size (int16-indexable)
TPH = 8            # gather tiles per half-window (dma_gather limit: 1024 idxs/call)
CAP = TPH * 128    # src slots per half-window
BIG = 1 << 20      # OOB scatter index (dropped by bounds_check)

_prog_cache = {}


def _pack_core(es, ed):
    """Pack one core's dst-sorted edges into windows.

    es: global src ids, ed: local dst ids (0..NLOC), both sorted by ed.
    Returns windows [(n0, n1, sA, dA, sB, dB)]: node range [n0,n1), per-half
    src ids and local dst ids (window edge lists, dst-sorted).
    """
    is_b = es >= HALF
    eA, dA = es[~is_b], ed[~is_b]
    eB, dB = es[is_b], ed[is_b]
    cumA = np.concatenate([[0], np.cumsum(np.bincount(dA, minlength=NLOC))])
    cumB = np.concatenate([[0], np.cumsum(np.bincount(dB, minlength=NLOC))])
    windows = []
    n0 = 0
    while n0 < NLOC:
        n1 = min(n0 + 128, NLOC)
        a_hi = int(np.searchsorted(cumA, cumA[n0] + CAP, side="right")) - 1
        b_hi = int(np.searchsorted(cumB, cumB[n0] + CAP, side="right")) - 1
        n1 = min(n1, a_hi, b_hi)
        if n1 <= n0:
            raise RuntimeError(f"node {n0} degree exceeds window capacity")
        windows.append((n0, n1,
                        eA[cumA[n0]:cumA[n1]], dA[cumA[n0]:cumA[n1]],
                        eB[cumB[n0]:cumB[n1]], dB[cumB[n0]:cumB[n1]]))
        n0 = n1
    return windows


def _wrap16(a):
    """[W, CAP] int16 -> [128, W*CAP/16] per-16 wrap, replicated x8."""
    Wn = a.shape[0]
    w16 = a.reshape(Wn, CAP // 16, 16).transpose(2, 0, 1).reshape(16, -1)
    return np.tile(w16, (8, 1)).copy()


def preprocess(x, edge_index, batch, W1, b1, W2, b2):
    src = np.asarray(edge_index[0], dtype=np.int64)
    dst = np.asarray(edge_index[1], dtype=np.int64)
    deg = np.bincount(dst, minlength=N).astype(np.float64) + 1.0
    dinv = (1.0 / np.sqrt(deg)).astype(np.float32)

    loop = np.arange(N, dtype=np.int64)          # self-loops as plain edges
    srcs = np.concatenate([src, loop])
    dsts = np.concatenate([dst, loop])

    x1p = np.zeros((N, DP), np.float32)
    x1p[:, :D] = dinv[:, None] * np.asarray(x, np.float32)

    batch_np = np.asarray(batch, np.int64)
    per_core_wins = []
    for k in range(NC):
        base = k * NLOC
        m = (dsts >= base) & (dsts < base + NLOC)
        es = srcs[m]
        ed = (dsts[m] - base).astype(np.int64)
        order = np.argsort(ed, kind="stable")
        per_core_wins.append(_pack_core(es[order], ed[order]))

    W = max(len(w) for w in per_core_wins)

    cores = []
    for k in range(NC):
        base = k * NLOC
        wins = per_core_wins[k]
        idxA = np.zeros((W, CAP), np.int16)
        idxB = np.zeros((W, CAP), np.int16)
        dstloc = np.full((W, 2 * CAP), -1.0, np.float32)
        sidx = np.full((W, 128), BIG, np.int32)
        dinvw = np.ones((W, 128), np.float32)
        batchg = np.full((W, 128), -1.0, np.float32)
        for w, (n0, n1, sA, dA, sB, dB) in enumerate(wins):
            nn = n1 - n0
            idxA[w, :len(sA)] = sA.astype(np.int16)
            idxB[w, :len(sB)] = (sB - HALF).astype(np.int16)
            dstloc[w, :len(dA)] = (dA - n0).astype(np.float32)
            dstloc[w, CAP:CAP + len(dB)] = (dB - n0).astype(np.float32)
            sidx[w, :nn] = np.arange(n0, n1, dtype=np.int32)
            dinvw[w, :nn] = dinv[base + np.arange(n0, n1)]
            batchg[w, :nn] = batch_np[base + np.arange(n0, n1)].astype(np.float32)
        # device layouts
        cores.append(dict(
            idxa=_wrap16(idxA),
            idxb=_wrap16(idxB),
            # slot i of window tile t -> [i%128, w*2*TPH + t]
            dstloc=dstloc.reshape(W, 2 * TPH, 128).transpose(2, 0, 1).reshape(128, W * 2 * TPH).copy(),
            sidx=sidx.T.copy(),          # [128, W]
            dinvw=dinvw.T.copy(),        # [128, W]
            batchg=batchg.T.copy(),      # [128, W]
        ))

    def wpad(Wm, bv):
        p = np.zeros((DP, DP), np.float32)
        p[:D, :D] = np.asarray(Wm, np.float32)
        p[DP - 1, :D] = np.asarray(bv, np.float32)   # bias via ones-slot 191
        return p[:128].copy(), p[128:].copy()        # [128,192], [64,192]

    wa1, wb1 = wpad(W1, b1)
    wa2, wb2 = wpad(W2, b2)
    consts = dict(
        iota=np.tile(np.arange(G, dtype=np.float32), (128, 1)),
        ident=np.eye(128, dtype=np.float32),
        wa1=wa1, wb1=wb1, wa2=wa2, wb2=wb2, x1p=x1p,
    )
    counts = np.bincount(batch_np, minlength=G).astype(np.float32)
    return cores, consts, W, counts


def build_program(W, use_collective=True, repeats=1):
    import concourse.bacc as bacc
    import concourse.bass as bass
    import concourse.mybir as mybir
    import concourse.tile as tile

    nc = bacc.Bacc("TRN2", target_bir_lowering=False, debug=False)
    dt = mybir.dt
    f32 = dt.float32

    x1p_d = nc.dram_tensor("x1p", [N, DP], f32, kind="ExternalInput")
    idxa_d = nc.dram_tensor("idxa", [128, W * CAP // 16], dt.int16, kind="ExternalInput")
    idxb_d = nc.dram_tensor("idxb", [128, W * CAP // 16], dt.int16, kind="ExternalInput")
    dstloc_d = nc.dram_tensor("dstloc", [128, W * 2 * TPH], f32, kind="ExternalInput")
    sidx_d = nc.dram_tensor("sidx", [128, W], dt.int32, kind="ExternalInput")
    dinvw_d = nc.dram_tensor("dinvw", [128, W], f32, kind="ExternalInput")
    batchg_d = nc.dram_tensor("batchg", [128, W], f32, kind="ExternalInput")
    iota_d = nc.dram_tensor("iota", [128, G], f32, kind="ExternalInput")
    ident_d = nc.dram_tensor("ident", [128, 128], f32, kind="ExternalInput")
    w_d = {nm: nc.dram_tensor(nm, [128 if nm[1] == "a" else 64, DP], f32,
                              kind="ExternalInput")
           for nm in ("wa1", "wb1", "wa2", "wb2")}
    pool_out = nc.dram_tensor("pool", [G, DP], f32, kind="ExternalOutput")

    t2full = nc.dram_tensor("t2full", [N, DP], f32)
    t2b = nc.dram_tensor("t2b", [NLOC, DP], f32)

    Relu = mybir.ActivationFunctionType.Relu
    Copy = mybir.ActivationFunctionType.Copy

    with tile.TileContext(nc) as tc:
        with (
            tc.tile_pool(name="const", bufs=1) as cpool,
            tc.tile_pool(name="work", bufs=3) as wpool,
            tc.tile_pool(name="oh", bufs=4) as ohpool,
            tc.tile_pool(name="ps_agg", bufs=2, space="PSUM") as ps_agg,
            tc.tile_pool(name="ps_tp", bufs=1, space="PSUM") as ps_tp,
            tc.tile_pool(name="ps_out", bufs=2, space="PSUM") as ps_out,
            tc.tile_pool(name="ps_pool", bufs=1, space="PSUM") as ps_pool,
        ):
            def cload(dram, shape, dtype=f32):
                t = cpool.tile(shape, dtype, name=f"c_{dram.name}",
                               tag=f"c_{dram.name}")
                nc.sync.dma_start(out=t[:], in_=dram[:])
                return t

            idxa = cload(idxa_d, [128, W * CAP // 16], dt.int16)
            idxb = cload(idxb_d, [128, W * CAP // 16], dt.int16)
            dstloc = cload(dstloc_d, [128, W * 2 * TPH])
            sidx = cload(sidx_d, [128, W], dt.int32)
            dinvw = cload(dinvw_d, [128, W])
            batchg = cload(batchg_d, [128, W])
            iota = cload(iota_d, [128, G])
            ident = cload(ident_d, [128, 128])
            wt = {nm: cload(w_d[nm], [128 if nm[1] == "a" else 64, DP])
                  for nm in ("wa1", "wb1", "wa2", "wb2")}

            pool_ps = [ps_pool.tile([128, DP], f32, space="PSUM", tag=f"pp{i}",
                                    name=f"pool_ps{i}")
                       for i in range(2)]

            # pre-zero t2b so unwritten rows (reduced-W timing builds) and the
            # AllGather never read uninitialized DRAM
            zt = cpool.tile([128, DP], f32, name="zt", tag="zt")
            nc.vector.memset(zt[:], 0.0)
            for r0 in range(0, NLOC, 128):
                rn = min(128, NLOC - r0)
                nc.sync.dma_start(out=t2b[r0:r0 + rn, :], in_=zt[:rn, :])

            for rep in range(repeats):
              for lam in (0, 1):
                tab = x1p_d if lam == 0 else t2full
                wa = wt["wa1"] if lam == 0 else wt["wa2"]
                wb = wt["wb1"] if lam == 0 else wt["wb2"]
                for w in range(W):
                    agg = ps_agg.tile([128, DP], f32, space="PSUM", tag="agg")
                    for h in (0, 1):
                        msg = wpool.tile([128, TPH, DP], f32, tag="msg")
                        idx_t = idxa if h == 0 else idxb
                        tab_ap = tab[0:HALF, :] if h == 0 else tab[HALF:N, :]
                        nc.gpsimd.dma_gather(
                            msg[:], tab_ap,
                            idx_t[:, w * (CAP // 16):(w + 1) * (CAP // 16)],
                            CAP, CAP, DP,
                        )
                        for t in range(TPH):
                            oh = ohpool.tile([128, 128], f32, tag="oh")
                            col = w * 2 * TPH + h * TPH + t
                            nc.vector.tensor_tensor(
                                out=oh[:],
                                in0=dstloc[:, col:col + 1].to_broadcast([128, 128]),
                                in1=iota[:, 0:128],
                                op=mybir.AluOpType.is_equal,
                            )
                            nc.tensor.matmul(
                                out=agg[:], lhsT=oh[:], rhs=msg[:, t, :],
                                start=(h == 0 and t == 0),
                                stop=(h == 1 and t == TPH - 1),
                            )
                    sT = wpool.tile([128, DP], f32, tag="sT")
                    nc.scalar.activation(out=sT[:], in_=agg[:], func=Copy,
                                         scale=dinvw[:, w:w + 1])
                    nc.vector.memset(sT[:, DP - 1:DP], 1.0)
                    tp1 = ps_tp.tile([128, 128], f32, space="PSUM", tag="tp1")
                    tp2 = ps_tp.tile([64, 128], f32, space="PSUM", tag="tp2")
                    nc.tensor.transpose(out=tp1[:], in_=sT[:, 0:128],
                                        identity=ident[:])
                    nc.tensor.transpose(out=tp2[:], in_=sT[:, 128:DP],
                                        identity=ident[:])
                    tT1 = wpool.tile([128, 128], f32, tag="tT1")
                    tT2 = wpool.tile([64, 128], f32, tag="tT2")
                    nc.scalar.activation(out=tT1[:], in_=tp1[:], func=Copy)
                    nc.scalar.activation(out=tT2[:], in_=tp2[:], func=Copy)
                    outp = ps_out.tile([128, DP], f32, space="PSUM", tag="outp")
                    nc.tensor.matmul(out=outp[:], lhsT=tT1[:], rhs=wa[:], start=True, stop=False)
                    nc.tensor.matmul(out=outp[:], lhsT=tT2[:], rhs=wb[:], start=False, stop=True)
                    if lam == 0:
                        tabt = wpool.tile([128, DP], f32, tag="tabt")
                        nc.scalar.activation(out=tabt[:], in_=outp[:], func=Relu,
                                             scale=dinvw[:, w:w + 1])
                        nc.gpsimd.indirect_dma_start(
                            out=t2b[:],
                            out_offset=bass.IndirectOffsetOnAxis(
                                ap=sidx[:, w:w + 1], axis=0),
                            in_=tabt[:],
                            in_offset=None,
                            bounds_check=NLOC - 1,
                            oob_is_err=False,
                        )
                    else:
                        h2 = wpool.tile([128, DP], f32, tag="tabt")
                        nc.scalar.activation(out=h2[:], in_=outp[:], func=Relu)
                        og = ohpool.tile([128, G], f32, tag="og")
                        nc.vector.tensor_tensor(
                            out=og[:],
                            in0=batchg[:, w:w + 1].to_broadcast([128, G]),
                            in1=iota[:],
                            op=mybir.AluOpType.is_equal,
                        )
                        for i in range(2):
                            nc.tensor.matmul(
                                out=pool_ps[i][:],
                                lhsT=og[:, 128 * i:128 * (i + 1)], rhs=h2[:],
                                start=(w == 0), stop=(w == W - 1),
                            )
                if lam == 0 and use_collective:
                    nc.gpsimd.collective_compute(
                        "AllGather", mybir.AluOpType.bypass,
                        replica_groups=[list(range(NC))],
                        ins=[t2b[:]], outs=[t2full[:]],
                    )
            for i in range(2):
                po = wpool.tile([128, DP], f32, tag="po")
                nc.scalar.activation(out=po[:], in_=pool_ps[i][:], func=Copy)
                nc.sync.dma_start(out=pool_out[128 * i:128 * (i + 1), :], in_=po[:])

    nc.compile()
    return nc


def kernel(**inputs):
    from concourse.bass_utils import run_bass_kernel_spmd

    cores, consts, W, counts = preprocess(**inputs)
    if W not in _prog_cache:
        _prog_cache[W] = build_program(W)
    nc = _prog_cache[W]

    in_maps = [{**consts, **{k2: v for k2, v in c.items()}} for c in cores]
    res = run_bass_kernel_spmd(nc, in_maps, core_ids=list(range(NC)))
    total = np.zeros((G, DP), np.float32)
    for c in range(NC):
        total += res.results[c]["pool"]
    out = total[:, :D] / np.maximum(counts, 1.0)[:, None]
    return out.astype(np.float32)
